# revision 1
# baseline (speedup 1.0000x reference)
"""RBF-kernel SVM inference on 8 Trainium2 NeuronCores.

out[m] = sum_n w[n] * exp(-g * ||x[m] - xt[n]||^2) + b
       = exp(-g*||x[m]||^2) * sum_n w[n] * exp(2g*x[m].xt[n] - g*||xt[n]||^2) + b

Sharding: rows of x split across 8 cores (1024 each); x_train / weight
replicated. Per core: a [8192, 1024] kernel slab via bf16 matmuls
(K=512 contraction, n on partitions), Exp on ScalarE with the
-g*||xt||^2 term as per-partition bias, and the weighted n-reduction as
M=1 matmuls with the w column as the stationary operand, accumulated in
PSUM across all 64 n-tiles.
"""

import os
import sys

for _p in ("/opt/trn_rl_repo", "/root/.axon_site/_ro/trn_rl_repo"):
    if os.path.isdir(_p) and _p not in sys.path:
        sys.path.append(_p)

import numpy as np
import ml_dtypes

import concourse.bass as bass
import concourse.mybir as mybir
import concourse.tile as tile
from concourse import bacc
from concourse.bass_utils import run_bass_kernel_spmd

M, N, D = 8192, 8192, 512
NCORES = 8
MC = M // NCORES  # rows of x per core

LAST_RESULTS = None  # BassKernelResults of the most recent run (for test.py)


def build(mc=MC, n=N, d=D, ncores=NCORES):
    """Build + compile the per-core program. Returns the Bacc instance."""
    P = 128
    KT = d // P            # K tiles in the contraction
    NT = n // P            # n tiles (partition dim of the kernel slab)
    MCH = min(512, mc)     # PSUM free-dim chunk (one f32 bank)
    MJ = mc // MCH         # m chunks
    NB = max(1, NT // 8)   # n tiles per DMA group of x_train
    NG = NT // NB          # number of DMA groups

    f32 = mybir.dt.float32
    bf16 = mybir.dt.bfloat16

    nc = bacc.Bacc(
        "TRN2",
        target_bir_lowering=False,
        debug=False,
        enable_asserts=False,
        num_devices=ncores,
    )

    xt_d = nc.dram_tensor("xt", (d, mc), bf16, kind="ExternalInput")
    bt_d = nc.dram_tensor("bt", (d, n), bf16, kind="ExternalInput")
    wc_d = nc.dram_tensor("wc", (P, NT), f32, kind="ExternalInput")
    bn_d = nc.dram_tensor("bn", (P, NT), f32, kind="ExternalInput")
    xx_d = nc.dram_tensor("xx", (1, mc), f32, kind="ExternalInput")
    bs_d = nc.dram_tensor("bs", (1, 1), f32, kind="ExternalInput")
    out_d = nc.dram_tensor("out", (1, mc), f32, kind="ExternalOutput")

    EXP = mybir.ActivationFunctionType.Exp

    with tile.TileContext(nc) as tc:
        with (
            tc.tile_pool(name="const", bufs=1) as const,
            tc.tile_pool(name="bt_pool", bufs=1) as bt_pool,
            tc.tile_pool(name="e_pool", bufs=3) as e_pool,
            tc.tile_pool(name="pt_pool", bufs=3, space="PSUM") as pt_pool,
            tc.tile_pool(name="ps_pool", bufs=1, space="PSUM") as ps_pool,
        ):
            wc_sb = const.tile([P, NT], f32, name="wc_sb")
            bn_sb = const.tile([P, NT], f32, name="bn_sb")
            xx_sb = const.tile([1, mc], f32, name="xx_sb")
            bs_sb = const.tile([1, 1], f32, name="bs_sb")
            xt_sb = const.tile([P, KT, mc], bf16, name="xt_sb")
            exmm = const.tile([1, mc], f32, name="exmm")
            fin = const.tile([1, mc], f32, name="fin")

            nc.sync.dma_start(wc_sb[:], wc_d[:])
            nc.sync.dma_start(bn_sb[:], bn_d[:])
            nc.sync.dma_start(xx_sb[:], xx_d[:])
            nc.sync.dma_start(bs_sb[:], bs_d[:])
            for k in range(KT):
                nc.sync.dma_start(xt_sb[:, k, :], xt_d[k * P:(k + 1) * P, :])

            # exp(-g*||x_m||^2) row, used in the final combine
            nc.scalar.activation(exmm[:], xx_sb[:], EXP)

            # x_train^T (pre-scaled by 2g on host), grouped for DMA/compute overlap
            bt_sb = []
            for g in range(NG):
                t = bt_pool.tile([P, KT, NB * P], bf16, name=f"bt_sb{g}")
                for k in range(KT):
                    nc.sync.dma_start(
                        t[:, k, :], bt_d[k * P:(k + 1) * P, g * NB * P:(g + 1) * NB * P]
                    )
                bt_sb.append(t)

            # S[1, mc] accumulators, one PSUM bank per m chunk
            ps = [ps_pool.tile([1, MCH], f32, name=f"ps{j}") for j in range(MJ)]

            # main pipeline over n tiles; the reduction matmul for tile nt is
            # emitted one iteration late so PE streams tile nt+1 while ScalarE
            # computes exp(t[nt])
            pend = None  # (e_tile, nt)
            for nt in range(NT):
                g, r = divmod(nt, NB)
                pt = pt_pool.tile([P, mc], f32, name="pt")
                for j in range(MJ):
                    for k in range(KT):
                        nc.tensor.matmul(
                            pt[:, j * MCH:(j + 1) * MCH],
                            bt_sb[g][:, k, r * P:(r + 1) * P],
                            xt_sb[:, k, j * MCH:(j + 1) * MCH],
                            start=(k == 0),
                            stop=(k == KT - 1),
                        )
                e = e_pool.tile([P, mc], f32, name="e")
                nc.scalar.activation(e[:], pt[:], EXP, bias=bn_sb[:, nt:nt + 1])

                if pend is not None:
                    pe, pnt = pend
                    for j in range(MJ):
                        nc.tensor.matmul(
                            ps[j][:],
                            wc_sb[:, pnt:pnt + 1],
                            pe[:, j * MCH:(j + 1) * MCH],
                            start=(pnt == 0),
                            stop=False,
                        )
                pend = (e, nt)

            pe, pnt = pend
            for j in range(MJ):
                nc.tensor.matmul(
                    ps[j][:],
                    wc_sb[:, pnt:pnt + 1],
                    pe[:, j * MCH:(j + 1) * MCH],
                    start=(pnt == 0),
                    stop=True,
                )

            # out = S * exp(-g*xx) + b
            for j in range(MJ):
                sl = slice(j * MCH, (j + 1) * MCH)
                nc.vector.tensor_mul(fin[:, sl], ps[j][:], exmm[:, sl])
                nc.vector.tensor_scalar_add(fin[:, sl], fin[:, sl], bs_sb[:])
            nc.sync.dma_start(out_d[:], fin[:])

    nc.compile()
    return nc


_CACHE = {}


def _get_nc():
    if "nc" not in _CACHE:
        _CACHE["nc"] = build()
    return _CACHE["nc"]


def kernel(x, x_train, gamma, weight, bias):
    global LAST_RESULTS
    x = np.asarray(x, dtype=np.float32)
    x_train = np.asarray(x_train, dtype=np.float32)
    g = float(np.asarray(gamma).reshape(-1)[0])
    w = np.asarray(weight, dtype=np.float32).reshape(N)
    b = np.float32(np.asarray(bias).reshape(-1)[0])

    NT = N // 128
    xx = np.einsum("md,md->m", x.astype(np.float64), x.astype(np.float64))
    yy = np.einsum("nd,nd->n", x_train.astype(np.float64), x_train.astype(np.float64))

    xt_all = np.ascontiguousarray(x.T).astype(ml_dtypes.bfloat16)          # [D, M]
    bt = np.ascontiguousarray((2.0 * g) * x_train.T).astype(ml_dtypes.bfloat16)  # [D, N]
    wc = np.ascontiguousarray(w.reshape(NT, 128).T).astype(np.float32)     # wc[p,nt]
    bn = np.ascontiguousarray((-g * yy).astype(np.float32).reshape(NT, 128).T)
    bs = np.full((1, 1), b, dtype=np.float32)
    xxn = (-g * xx).astype(np.float32)

    in_maps = []
    for c in range(NCORES):
        sl = slice(c * MC, (c + 1) * MC)
        in_maps.append({
            "xt": np.ascontiguousarray(xt_all[:, sl]),
            "bt": bt,
            "wc": wc,
            "bn": bn,
            "xx": xxn[sl].reshape(1, MC),
            "bs": bs,
        })

    nc = _get_nc()
    res = run_bass_kernel_spmd(nc, in_maps, core_ids=list(range(NCORES)))
    LAST_RESULTS = res
    out = np.concatenate(
        [np.asarray(res.results[c]["out"], dtype=np.float32).reshape(MC) for c in range(NCORES)]
    )
    return out.reshape(M, 1)


# revision 5
# speedup vs baseline: 1.4564x; 1.4564x over previous
"""RBF-kernel SVM inference on 8 Trainium2 NeuronCores.

out[m] = sum_n w[n] * exp(-g * ||x[m] - xt[n]||^2) + b
       = exp(-g*||x[m]||^2) * sum_n w[n] * exp(2g*x[m].xt[n] - g*||xt[n]||^2) + b

Sharding: rows of x split across 8 cores (1024 each); x_train / weight
replicated. Per core: a [8192, 1024] kernel slab via bf16 matmuls
(K=512 contraction, n on partitions), Exp on ScalarE with the
-g*||xt||^2 term as per-partition bias, and the weighted n-reduction as
M=1 matmuls with the w column as the stationary operand, accumulated in
PSUM across all 64 n-tiles.
"""

import os
import sys

for _p in ("/opt/trn_rl_repo", "/root/.axon_site/_ro/trn_rl_repo"):
    if os.path.isdir(_p) and _p not in sys.path:
        sys.path.append(_p)

import numpy as np
import ml_dtypes

import concourse.bass as bass
import concourse.mybir as mybir
import concourse.tile as tile
from concourse import bacc
from concourse.bass_utils import run_bass_kernel_spmd

M, N, D = 8192, 8192, 512
NCORES = 8
MC = M // NCORES  # rows of x per core

LAST_RESULTS = None  # BassKernelResults of the most recent run (for test.py)


def build(mc=MC, n=N, d=D, ncores=NCORES):
    """Build + compile the per-core program. Returns the Bacc instance."""
    P = 128
    KT = d // P            # K tiles in the contraction
    NT = n // P            # n tiles (partition dim of the kernel slab)
    MCH = min(512, mc)     # PSUM free-dim chunk (one f32 bank)
    MJ = mc // MCH         # m chunks
    NB = max(1, NT // 8)   # n tiles per DMA group of x_train
    NG = NT // NB          # number of DMA groups

    f32 = mybir.dt.float32
    bf16 = mybir.dt.bfloat16

    nc = bacc.Bacc(
        "TRN2",
        target_bir_lowering=False,
        debug=False,
        enable_asserts=False,
        num_devices=ncores,
    )

    xt_d = nc.dram_tensor("xt", (d, mc), bf16, kind="ExternalInput")
    bt_d = nc.dram_tensor("bt", (d, n), bf16, kind="ExternalInput")
    wc_d = nc.dram_tensor("wc", (P, NT), bf16, kind="ExternalInput")
    bn_d = nc.dram_tensor("bn", (P, NT), f32, kind="ExternalInput")
    xx_d = nc.dram_tensor("xx", (1, mc), f32, kind="ExternalInput")
    bs_d = nc.dram_tensor("bs", (1, 1), f32, kind="ExternalInput")
    out_d = nc.dram_tensor("out", (1, mc), f32, kind="ExternalOutput")

    EXP = mybir.ActivationFunctionType.Exp

    with tile.TileContext(nc) as tc:
        with (
            tc.tile_pool(name="const", bufs=1) as const,
            tc.tile_pool(name="bt_pool", bufs=1) as bt_pool,
            tc.tile_pool(name="e_pool", bufs=3) as e_pool,
            tc.tile_pool(name="pt_pool", bufs=3, space="PSUM") as pt_pool,
            tc.tile_pool(name="ps_pool", bufs=1, space="PSUM") as ps_pool,
        ):
            wc_sb = const.tile([P, NT], bf16, name="wc_sb")
            bn_sb = const.tile([P, NT], f32, name="bn_sb")
            xx_sb = const.tile([1, mc], f32, name="xx_sb")
            bs_sb = const.tile([1, 1], f32, name="bs_sb")
            xt_sb = const.tile([P, KT, mc], bf16, name="xt_sb")
            exmm = const.tile([1, mc], f32, name="exmm")
            fin = const.tile([1, mc], f32, name="fin")

            nc.sync.dma_start(wc_sb[:], wc_d[:])
            nc.sync.dma_start(bn_sb[:], bn_d[:])
            nc.sync.dma_start(xx_sb[:], xx_d[:])
            nc.sync.dma_start(bs_sb[:], bs_d[:])
            for k in range(KT):
                nc.sync.dma_start(xt_sb[:, k, :], xt_d[k * P:(k + 1) * P, :])

            # exp(-g*||x_m||^2) row, used in the final combine
            nc.scalar.activation(exmm[:], xx_sb[:], EXP)

            # x_train^T (pre-scaled by 2g on host), grouped for DMA/compute overlap
            bt_sb = []
            for g in range(NG):
                t = bt_pool.tile([P, KT, NB * P], bf16, name=f"bt_sb{g}")
                for k in range(KT):
                    nc.sync.dma_start(
                        t[:, k, :], bt_d[k * P:(k + 1) * P, g * NB * P:(g + 1) * NB * P]
                    )
                bt_sb.append(t)

            # S[1, mc] accumulators, one PSUM bank per m chunk
            ps = [ps_pool.tile([1, MCH], f32, name=f"ps{j}") for j in range(MJ)]

            # main pipeline over n tiles; the reduction matmul for tile nt is
            # emitted one iteration late so PE streams tile nt+1 while ScalarE
            # computes exp(t[nt])
            pend = None  # (e_tile, nt)
            for nt in range(NT):
                g, r = divmod(nt, NB)
                pt = pt_pool.tile([P, mc], f32, name="pt")
                for k in range(KT):
                    for j in range(MJ):
                        nc.tensor.matmul(
                            pt[:, j * MCH:(j + 1) * MCH],
                            bt_sb[g][:, k, r * P:(r + 1) * P],
                            xt_sb[:, k, j * MCH:(j + 1) * MCH],
                            start=(k == 0),
                            stop=(k == KT - 1),
                        )
                e = e_pool.tile([P, mc], bf16, name="e")
                nc.scalar.activation(e[:], pt[:], EXP, bias=bn_sb[:, nt:nt + 1])

                if pend is not None:
                    pe, pnt = pend
                    for j in range(MJ):
                        nc.tensor.matmul(
                            ps[j][:],
                            wc_sb[:, pnt:pnt + 1],
                            pe[:, j * MCH:(j + 1) * MCH],
                            start=(pnt == 0),
                            stop=False,
                        )
                pend = (e, nt)

            pe, pnt = pend
            for j in range(MJ):
                nc.tensor.matmul(
                    ps[j][:],
                    wc_sb[:, pnt:pnt + 1],
                    pe[:, j * MCH:(j + 1) * MCH],
                    start=(pnt == 0),
                    stop=True,
                )

            # out = S * exp(-g*xx) + b
            for j in range(MJ):
                sl = slice(j * MCH, (j + 1) * MCH)
                nc.vector.tensor_mul(fin[:, sl], ps[j][:], exmm[:, sl])
                nc.vector.tensor_scalar_add(fin[:, sl], fin[:, sl], bs_sb[:])
            nc.sync.dma_start(out_d[:], fin[:])

    nc.compile()
    return nc


_CACHE = {}


def _get_nc():
    if "nc" not in _CACHE:
        _CACHE["nc"] = build()
    return _CACHE["nc"]


def kernel(x, x_train, gamma, weight, bias):
    global LAST_RESULTS
    x = np.asarray(x, dtype=np.float32)
    x_train = np.asarray(x_train, dtype=np.float32)
    g = float(np.asarray(gamma).reshape(-1)[0])
    w = np.asarray(weight, dtype=np.float32).reshape(N)
    b = np.float32(np.asarray(bias).reshape(-1)[0])

    NT = N // 128
    xx = np.einsum("md,md->m", x.astype(np.float64), x.astype(np.float64))
    yy = np.einsum("nd,nd->n", x_train.astype(np.float64), x_train.astype(np.float64))

    xt_all = np.ascontiguousarray(x.T).astype(ml_dtypes.bfloat16)          # [D, M]
    bt = np.ascontiguousarray((2.0 * g) * x_train.T).astype(ml_dtypes.bfloat16)  # [D, N]
    wc = np.ascontiguousarray(w.reshape(NT, 128).T).astype(ml_dtypes.bfloat16)  # wc[p,nt]
    bn = np.ascontiguousarray((-g * yy).astype(np.float32).reshape(NT, 128).T)
    bs = np.full((1, 1), b, dtype=np.float32)
    xxn = (-g * xx).astype(np.float32)

    in_maps = []
    for c in range(NCORES):
        sl = slice(c * MC, (c + 1) * MC)
        in_maps.append({
            "xt": np.ascontiguousarray(xt_all[:, sl]),
            "bt": bt,
            "wc": wc,
            "bn": bn,
            "xx": xxn[sl].reshape(1, MC),
            "bs": bs,
        })

    nc = _get_nc()
    res = run_bass_kernel_spmd(nc, in_maps, core_ids=list(range(NCORES)))
    LAST_RESULTS = res
    out = np.concatenate(
        [np.asarray(res.results[c]["out"], dtype=np.float32).reshape(MC) for c in range(NCORES)]
    )
    return out.reshape(M, 1)


# revision 11
# speedup vs baseline: 2.0518x; 1.4088x over previous
"""RBF-kernel SVM inference on 8 Trainium2 NeuronCores.

out[m] = sum_n w[n] * exp(-g * ||x[m] - xt[n]||^2) + b
       = exp(-g*||x[m]||^2) * sum_n w[n] * exp(2g*x[m].xt[n] - g*||xt[n]||^2) + b

Sharding: rows of x split across 8 cores (1024 each); x_train / weight
replicated. Per core: a [8192, 1024] kernel slab via bf16 matmuls
(K=512 contraction, n on partitions), Exp on ScalarE with the
-g*||xt||^2 term as per-partition bias, and the weighted n-reduction as
M=1 matmuls with the w column as the stationary operand, accumulated in
PSUM across all 64 n-tiles.
"""

import os
import sys

for _p in ("/opt/trn_rl_repo", "/root/.axon_site/_ro/trn_rl_repo"):
    if os.path.isdir(_p) and _p not in sys.path:
        sys.path.append(_p)

import numpy as np
import ml_dtypes

import concourse.bass as bass
import concourse.mybir as mybir
import concourse.tile as tile
from concourse import bacc
from concourse.bass_utils import run_bass_kernel_spmd

M, N, D = 8192, 8192, 512
NCORES = 8
MC = M // NCORES  # rows of x per core

LAST_RESULTS = None  # BassKernelResults of the most recent run (for test.py)


def build(mc=MC, n=N, d=D, ncores=NCORES):
    """Build + compile the per-core program. Returns the Bacc instance."""
    P = 128
    KT = d // P            # K tiles in the contraction
    NT = n // P            # n tiles (partition dim of the kernel slab)
    MCH = min(512, mc)     # PSUM free-dim chunk (one f32 bank)
    MJ = mc // MCH         # m chunks
    NB = max(1, NT // 8)   # n tiles per DMA group of x_train
    NG = NT // NB          # number of DMA groups

    f32 = mybir.dt.float32
    bf16 = mybir.dt.bfloat16
    f8 = mybir.dt.float8e4
    KP = KT // 2  # DoubleRow passes (K=256 each)

    nc = bacc.Bacc(
        "TRN2",
        target_bir_lowering=False,
        debug=False,
        enable_asserts=False,
        num_devices=ncores,
    )

    xt_d = nc.dram_tensor("xt", (d, mc), f8, kind="ExternalInput")
    bt_d = nc.dram_tensor("bt", (d, n), f8, kind="ExternalInput")
    wc_d = nc.dram_tensor("wc", (P, NT), bf16, kind="ExternalInput")
    bn_d = nc.dram_tensor("bn", (P, NT), f32, kind="ExternalInput")
    xx_d = nc.dram_tensor("xx", (1, mc), f32, kind="ExternalInput")
    bs_d = nc.dram_tensor("bs", (1, 1), f32, kind="ExternalInput")
    out_d = nc.dram_tensor("out", (1, mc), f32, kind="ExternalOutput")

    EXP = mybir.ActivationFunctionType.Exp

    with tile.TileContext(nc) as tc:
        with (
            tc.tile_pool(name="const", bufs=1) as const,
            tc.tile_pool(name="bt_pool", bufs=1) as bt_pool,
            tc.tile_pool(name="e_pool", bufs=3) as e_pool,
            tc.tile_pool(name="pt_pool", bufs=3, space="PSUM") as pt_pool,
            tc.tile_pool(name="ps_pool", bufs=1, space="PSUM") as ps_pool,
        ):
            wc_sb = const.tile([P, NT], bf16, name="wc_sb")
            bn_sb = const.tile([P, NT], f32, name="bn_sb")
            xx_sb = const.tile([1, mc], f32, name="xx_sb")
            bs_sb = const.tile([1, 1], f32, name="bs_sb")
            xt_sb = const.tile([P, KT, mc], f8, name="xt_sb")
            exmm = const.tile([1, mc], f32, name="exmm")
            fin = const.tile([1, mc], f32, name="fin")

            # x slab first (every matmul needs it), then the first x_train
            # group, then the small constants; remaining groups follow and
            # overlap with compute. Spread over three DGE queues.
            for k in range(KT):
                nc.sync.dma_start(xt_sb[:, k, :], xt_d[k * P:(k + 1) * P, :])

            bt_sb = []
            for g in range(NG):
                t = bt_pool.tile([P, KT, NB * P], f8, name=f"bt_sb{g}")
                bt_sb.append(t)

            def load_bt(g, eng):
                for k in range(KT):
                    eng.dma_start(
                        bt_sb[g][:, k, :],
                        bt_d[k * P:(k + 1) * P, g * NB * P:(g + 1) * NB * P],
                    )

            load_bt(0, nc.gpsimd)
            nc.sync.dma_start(wc_sb[:], wc_d[:])
            nc.sync.dma_start(bn_sb[:], bn_d[:])
            nc.sync.dma_start(xx_sb[:], xx_d[:])
            nc.sync.dma_start(bs_sb[:], bs_d[:])

            # exp(-g*||x_m||^2) row, used in the final combine
            nc.scalar.activation(exmm[:], xx_sb[:], EXP)

            dma_engs = [nc.sync, nc.gpsimd, nc.scalar]
            for g in range(1, NG):
                load_bt(g, dma_engs[g % len(dma_engs)])

            # S[1, mc] accumulators, one PSUM bank per m chunk
            ps = [ps_pool.tile([1, MCH], f32, name=f"ps{j}") for j in range(MJ)]

            # main pipeline over n tiles; the reduction matmul for tile nt is
            # emitted one iteration late so PE streams tile nt+1 while ScalarE
            # computes exp(t[nt])
            pend = None  # (e_tile, nt)
            for nt in range(NT):
                g, r = divmod(nt, NB)
                pt = pt_pool.tile([P, mc], f32, name="pt")
                for p in range(KP):
                    for j in range(MJ):
                        nc.tensor.matmul(
                            pt[:, j * MCH:(j + 1) * MCH],
                            bt_sb[g][:, 2 * p:2 * p + 2, r * P:(r + 1) * P],
                            xt_sb[:, 2 * p:2 * p + 2, j * MCH:(j + 1) * MCH],
                            start=(p == 0),
                            stop=(p == KP - 1),
                            perf_mode=mybir.MatmulPerfMode.DoubleRow,
                        )
                e = e_pool.tile([P, mc], bf16, name="e")
                nc.scalar.activation(e[:], pt[:], EXP, bias=bn_sb[:, nt:nt + 1])

                if pend is not None:
                    pe, pnt = pend
                    for j in range(MJ):
                        nc.tensor.matmul(
                            ps[j][:],
                            wc_sb[:, pnt:pnt + 1],
                            pe[:, j * MCH:(j + 1) * MCH],
                            start=(pnt == 0),
                            stop=False,
                        )
                pend = (e, nt)

            pe, pnt = pend
            for j in range(MJ):
                nc.tensor.matmul(
                    ps[j][:],
                    wc_sb[:, pnt:pnt + 1],
                    pe[:, j * MCH:(j + 1) * MCH],
                    start=(pnt == 0),
                    stop=True,
                )

            # out = S * exp(-g*xx) + b
            for j in range(MJ):
                sl = slice(j * MCH, (j + 1) * MCH)
                nc.vector.tensor_mul(fin[:, sl], ps[j][:], exmm[:, sl])
                nc.vector.tensor_scalar_add(fin[:, sl], fin[:, sl], bs_sb[:])
            nc.sync.dma_start(out_d[:], fin[:])

    nc.compile()
    return nc


_CACHE = {}


def _get_nc():
    if "nc" not in _CACHE:
        _CACHE["nc"] = build()
    return _CACHE["nc"]


def kernel(x, x_train, gamma, weight, bias):
    global LAST_RESULTS
    x = np.asarray(x, dtype=np.float32)
    x_train = np.asarray(x_train, dtype=np.float32)
    g = float(np.asarray(gamma).reshape(-1)[0])
    w = np.asarray(weight, dtype=np.float32).reshape(N)
    b = np.float32(np.asarray(bias).reshape(-1)[0])

    NT = N // 128
    xx = np.einsum("md,md->m", x.astype(np.float64), x.astype(np.float64))
    yy = np.einsum("nd,nd->n", x_train.astype(np.float64), x_train.astype(np.float64))

    xt_all = np.ascontiguousarray(x.T).astype(ml_dtypes.float8_e4m3)          # [D, M]
    bt = np.ascontiguousarray((2.0 * g) * x_train.T).astype(ml_dtypes.float8_e4m3)  # [D, N]
    wc = np.ascontiguousarray(w.reshape(NT, 128).T).astype(ml_dtypes.bfloat16)  # wc[p,nt]
    bn = np.ascontiguousarray((-g * yy).astype(np.float32).reshape(NT, 128).T)
    bs = np.full((1, 1), b, dtype=np.float32)
    xxn = (-g * xx).astype(np.float32)

    in_maps = []
    for c in range(NCORES):
        sl = slice(c * MC, (c + 1) * MC)
        in_maps.append({
            "xt": np.ascontiguousarray(xt_all[:, sl]),
            "bt": bt,
            "wc": wc,
            "bn": bn,
            "xx": xxn[sl].reshape(1, MC),
            "bs": bs,
        })

    nc = _get_nc()
    res = run_bass_kernel_spmd(nc, in_maps, core_ids=list(range(NCORES)))
    LAST_RESULTS = res
    out = np.concatenate(
        [np.asarray(res.results[c]["out"], dtype=np.float32).reshape(MC) for c in range(NCORES)]
    )
    return out.reshape(M, 1)
